# revision 26
# baseline (speedup 1.0000x reference)
"""FFNN-Transducer joint-lattice kernel for 8 Trainium2 NeuronCores.

Sorted-span-row decomposition: the unit of work is a 16-frame encoder
span (b, ts). All spans of the batch are sorted by target length
(usz+1) descending and packed into rows of 64 spans (8 cores x 8
spans); each row's joint width W is the max usz+1 within the row, so
u-padding adapts to the data with no fixed phase structure. Spans from
different samples share a row: the combined lhsT carries, per span, 16
encoder-projection rows plus that span's own prediction-bias rows, so
blocks mix samples freely and the t-axis is packed at 16-frame
granularity. The final partial row runs with TB = 16*ceil(left/8)
frames; its joint matmul is emitted v-partitioned (out[v, (t,u)]) so
the drain tail stays ~1us.

Per tile the device computes
    out[t,u,:] = tanh(enc_proj[t,:] + pred_bias[u,:]) @ jw2
with the tiny prediction network and the encoder projection enc@jw1[:E]
(~0.5% of FLOPs) on host.

Device pipeline per core, per row:
  PE:   per CH-frame chunk one "selection" matmul materializing
        A[j,(t,u)] = enc_proj[t,j] + bias[u,j] in PSUM; per u one
        [128x128] x [128x88] joint matmul (t-form), or jw2-stationary
        [128x88]^T @ hid for the small tail row (v-form).
  ACT:  batched tanh PSUM->SBUF fp16, one op per 3-bank A tile. A
        warmup tanh at t=0 preloads the activation table during the
        initial DMAs.
  DVE:  PSUM->SBUF evacuation (fp32->fp16); back-emission is paced by
        modeled ACT/DVE time so neither engine starves at row edges.
  DMA:  outputs streamed per u-group pair on the sync queue; selection
        matrices ride the otherwise-idle gpsimd (SWDGE) queue.

TRN2 fp32 matmul runs at 1/4 rate, so all TensorE-facing tensors are
fp16; PSUM stays fp32. jb2-add and the ragged scatter are host
epilogues.
"""

import os
import sys

for _p in ("/opt/trn_rl_repo", "/root/.axon_site/_ro/trn_rl_repo"):
    if os.path.isdir(_p) and _p not in sys.path:
        sys.path.append(_p)

import numpy as np

import concourse.bass as bass
import concourse.tile as tile
from concourse import bacc, mybir
from concourse.bass_utils import run_bass_kernel_spmd

# Problem dims (hardcoded per contract)
B, T, E = 8, 1000, 512
U = 100
U1 = U + 1
H, D, P = 2, 256, 256
J, V = 128, 88
BLANK = V - 1
N_CORES = 8

SPAN = 16           # t-frames per span (lhsT packing unit)
SPB = 8             # spans per core per full row (TB = 128)
UG = 5              # u-steps per M-PSUM bank in t-form backs

F32 = mybir.dt.float32
F16 = mybir.dt.float16

HID_BUFS = 4        # hid tile rotation depth (SBUF)
STG_BUFS = 3        # staging tile rotation depth (SBUF)
SPLIT_MIN_REST = 24  # u-split wide rows only if remainder >= this

_CACHE = {}


def _ch_for(w):
    """Largest divisor of SPAN with ch*w <= 512 (PSUM-bank column limit)."""
    ch = SPAN
    while ch > 1 and ch * w > 512:
        ch //= 2
    return ch


def _fronts_for(w, tb):
    """(t_off, n_t) A-tile steps covering tb frames, <=3 chunks of CH
    each, sized as evenly as CH granularity allows: a tiny final tile
    lets ACT overtake PE at section boundaries and exposes the PE->ACT
    semaphore latency."""
    ch = _ch_for(w)
    ft = 3 * ch
    nf = (tb + ft - 1) // ft
    nch_total = tb // ch
    steps, t = [], 0
    for i in range(nf):
        nch = (nch_total * (i + 1)) // nf - (nch_total * i) // nf
        n = nch * ch
        steps.append((t, n))
        t += n
    assert t == tb
    return steps


def _plan(tsz, usz):
    """Sort spans by target length, pack into rows of 64.

    Returns (U1e, rows); rows = [(TB, W, vform, assign)] where assign
    is the per-core list of (sample, t_start) spans (-1 = dummy).
    """
    usz1 = usz + 1
    spans = [(b, ts) for b in range(B) for ts in range(0, int(tsz[b]), SPAN)]
    if not spans:
        return None
    spans.sort(key=lambda s: (-int(usz1[s[0]]), s[0], s[1]))
    per_row = N_CORES * SPB
    rows = []
    i = 0
    while i < len(spans):
        chunk = spans[i:i + per_row]
        spr = (len(chunk) + N_CORES - 1) // N_CORES
        W = int(usz1[chunk[0][0]])
        chunk = chunk + [(-1, 0)] * (N_CORES * spr - len(chunk))
        assign = [chunk[c * spr:(c + 1) * spr] for c in range(N_CORES)]
        TB = spr * SPAN
        vform = TB < V  # short rows evacuate fewer cols v-partitioned
        rows.append((TB, W, vform, assign))
        i += per_row
    return int(usz1.max()), rows


def _sections(rows):
    """Split wide rows at u=64 so A-chunks fill PSUM banks (CH stays >= 8,
    fewer/larger ACT ops). Each section shares its row's comb lhsT; the
    selection matrix picks the section's bias rows.

    Returns [(row_id, TB, Wrow, u0, Ws, vform)].
    """
    secs = []
    for r, (TB, W, vform, _assign) in enumerate(rows):
        if W > 64 and not vform:
            # (W1<=64, W2<=32) halves CH-chunk count: 6+3 ACT ops per
            # 128 frames instead of 11. Requires the evened A-tile
            # fronts + capped back-drain, else boundary friction eats
            # the gain.
            w2 = min(32, W - 33)
            secs.append((r, TB, W, 0, W - w2, False))
            secs.append((r, TB, W, W - w2, w2, False))
        else:
            secs.append((r, TB, W, 0, W, vform))
    return secs


def _build_program(reps=1, geom=None):
    if geom is None:
        geom = _CACHE["geom"]
    # geom: tuple of (row_id, TB, Wrow, u0, Ws, vform) per section
    nc = bacc.Bacc("TRN2", target_bir_lowering=False, debug=False)

    jw2d = nc.dram_tensor("jw2d", [J, V], F16, kind="ExternalInput").ap()
    row_ids = []
    row_dims = {}
    sel_keys = []
    outd = []
    for p, (r, TBp, Wr, u0, Ws, vf) in enumerate(geom):
        if r not in row_ids:
            row_ids.append(r)
            row_dims[r] = (TBp, Wr)
        key = (Wr, u0, Ws)
        if key not in sel_keys:
            sel_keys.append(key)
        oshape = [V, TBp * Ws] if vf else [TBp, Ws * V]
        outd.append(nc.dram_tensor(
            f"out{p}", oshape, F16, kind="ExternalOutput").ap())
    combd = {r: nc.dram_tensor(
        f"combd{r}", [SPAN + row_dims[r][1], (row_dims[r][0] // SPAN) * J],
        F16, kind="ExternalInput").ap() for r in row_ids}
    seld = {k: nc.dram_tensor(f"selw{k[0]}_{k[1]}_{k[2]}",
                              [SPAN + k[0], SPAN * k[2]], F16,
                              kind="ExternalInput").ap()
            for k in sel_keys}

    hid_max = max(TBp * Ws for (_r, TBp, _Wr, _u0, Ws, _vf) in geom)
    stg_max = max((Ws * V if not vf else TBp * Ws)
                  for (_r, TBp, _Wr, _u0, Ws, vf) in geom)

    with tile.TileContext(nc) as tc:
        with (
            tc.tile_pool(name="singles", bufs=1) as singles,
            tc.tile_pool(name="hidp", bufs=HID_BUFS) as hidp,
            tc.tile_pool(name="stgp", bufs=STG_BUFS) as stgp,
            tc.tile_pool(name="psA", bufs=2, space="PSUM") as psA,
            tc.tile_pool(name="psM", bufs=2, space="PSUM") as psM,
        ):
            # warmup: preload the tanh table set while the first DMAs fly
            warm = singles.tile([128, 2], F16, tag="warm", name="warm")
            warm2 = singles.tile([128, 2], F16, tag="warm2", name="warm2")
            nc.vector.memset(warm[:, :], 0.0)
            nc.scalar.activation(out=warm2[:, :], in_=warm[:, :],
                                 func=mybir.ActivationFunctionType.Tanh)

            comb_sb = {}
            sel_sb = {}
            for r in row_ids:
                TBr, Wr = row_dims[r]
                comb_sb[r] = singles.tile(
                    [SPAN + Wr, (TBr // SPAN) * J], F16, tag=f"comb{r}",
                    name=f"comb_t{r}")
            for k in sel_keys:
                sel_sb[k] = singles.tile([SPAN + k[0], SPAN * k[2]], F16,
                                         tag=f"selw{k}", name=f"sel_t{k}")
            jw2_sb = singles.tile([J, V], F16, tag="jw2", name="jw2_sb")

            # first row's inputs first (parallel queues: comb on sync,
            # sel on the idle gpsimd/SWDGE queue), then everything else;
            # the first span's lhsT slice leads so the pipeline starts
            # after one tiny DMA instead of the full row load
            r0 = row_ids[0]
            nc.sync.dma_start(out=comb_sb[r0][:, 0:J], in_=combd[r0][:, 0:J])
            nc.sync.dma_start(out=comb_sb[r0][:, J:], in_=combd[r0][:, J:])
            nc.gpsimd.dma_start(out=sel_sb[sel_keys[0]][:, :],
                                in_=seld[sel_keys[0]][:, :])
            nc.sync.dma_start(out=jw2_sb[:, :], in_=jw2d[:, :])
            for k in sel_keys[1:]:
                nc.gpsimd.dma_start(out=sel_sb[k][:, :], in_=seld[k][:, :])
            for r in row_ids[1:]:
                nc.sync.dma_start(out=comb_sb[r][:, :], in_=combd[r][:, :])

            for rep in range(reps):
                _emit_rep(nc, hidp, stgp, psA, psM, comb_sb, jw2_sb, sel_sb,
                          outd, rep, geom, hid_max, stg_max,
                          last_rep=(rep == reps - 1))

    nc.compile()
    return nc


def _emit_rep(nc, hidp, stgp, psA, psM, comb_sb, jw2_sb, sel_sb, outd, rep,
              geom, hid_max, stg_max, last_rep=True):
    fronts = [_fronts_for(Ws, TBp) for (_r, TBp, _Wr, _u0, Ws, _vf) in geom]
    chs = [_ch_for(Ws) for (_r, _TB, _Wr, _u0, Ws, _vf) in geom]

    hid_t, stg_t, A_t = {}, {}, {}

    def front_mm(p, fi):
        r, TBp, Wr, u0, Ws, _vf = geom[p]
        CH = chs[p]
        t_off, n_t = fronts[p][fi]
        if fi == 0:
            hid_t[p] = hidp.tile([128, hid_max], F16, tag="hid",
                                 name=f"hid{rep}_{p}")
        A = psA.tile([128, 1536], F32, tag="A", name=f"A{rep}_{p}_{fi}")
        A_t[(p, fi)] = A
        sel = sel_sb[(Wr, u0, Ws)]
        for c in range(n_t // CH):
            tg = t_off + c * CH
            sp, tl = tg // SPAN, tg % SPAN
            nc.tensor.matmul(
                A[:, c * 512:c * 512 + CH * Ws],
                comb_sb[r][:, sp * J:(sp + 1) * J],
                sel[:, tl * Ws:(tl + CH) * Ws],
                start=True,
                stop=True,
            )

    def front_tanh(p, fi):
        _r, _TBp, _Wr, _u0, Ws, _vf = geom[p]
        CH = chs[p]
        t_off, n_t = fronts[p][fi]
        nch = n_t // CH
        A = A_t.pop((p, fi))
        nc.scalar.activation(
            out=hid_t[p][:, t_off * Ws:(t_off + n_t) * Ws].rearrange(
                "p (c x) -> p c x", c=nch),
            in_=A.rearrange("p (c x) -> p c x", c=3)[:, 0:nch, 0:CH * Ws],
            func=mybir.ActivationFunctionType.Tanh,
        )

    def n_backs(p):
        _r, TBp, _Wr, _u0, Ws, vf = geom[p]
        return ((TBp * Ws + 511) // 512) if vf else ((Ws + UG - 1) // UG)

    def back_cost(p, bi):
        _r, TBp, _Wr, _u0, Ws, vf = geom[p]
        if vf:
            n = min(512, TBp * Ws - bi * 512)
            return n * 1.05 + 130
        n_u = min(UG, Ws - bi * UG)
        return n_u * V * 1.05 + 130

    def back(p, bi, use_scalar):
        _r, TBp, _Wr, _u0, Ws, vf = geom[p]
        hid2 = hid_t[p]
        if bi == 0:
            stg_t[p] = stgp.tile([128, stg_max], F16, tag="stg",
                                 name=f"stg{rep}_{p}")
        stg = stg_t[p]
        M = psM.tile([128, 512], F32, tag="M", name=f"M{rep}_{p}_{bi}")
        cp = nc.scalar.copy if use_scalar else nc.vector.tensor_copy
        if vf:
            c0 = bi * 512
            n = min(512, TBp * Ws - c0)
            nc.tensor.matmul(
                M[0:V, 0:n],
                jw2_sb[:, :],
                hid2[:, c0:c0 + n],
                start=True,
                stop=True,
            )
            cp(out=stg[0:V, c0:c0 + n], in_=M[0:V, 0:n])
            if (bi + 1) * 512 >= TBp * Ws:
                nc.sync.dma_start(out=outd[p][:, :],
                                  in_=stg[0:V, 0:TBp * Ws])
            return
        NUGp = n_backs(p)
        ug0 = bi * UG
        n_u = min(UG, Ws - ug0)
        hid_ut = hid2[:, 0:TBp * Ws].rearrange("p (t u) -> p u t", u=Ws)
        for i in range(n_u):
            nc.tensor.matmul(
                M[0:TBp, i * V:(i + 1) * V],
                hid_ut[:, ug0 + i, :],
                jw2_sb[:, :],
                start=True,
                stop=True,
            )
        cp(out=stg[0:TBp, ug0 * V:(ug0 + n_u) * V], in_=M[0:TBp, 0:n_u * V])
        if bi % 2 == 1 or bi == NUGp - 1:
            u_lo = (bi // 2) * 2 * UG
            u_hi = ug0 + n_u
            nc.sync.dma_start(
                out=outd[p][:, u_lo * V:u_hi * V],
                in_=stg[0:TBp, u_lo * V:u_hi * V],
            )

    # software-pipelined emission: backs are paced against fronts by
    # modeled engine time (ACT ns for fronts, DVE ns for backs) so the
    # DVE lags ACT by a constant fraction across width changes
    def act_cost(p, fi):
        _t, n_t = fronts[p][fi]
        return n_t * geom[p][4] * 0.8333 + 190.0

    total_act = sum(act_cost(p, fi)
                    for p in range(len(geom)) for fi in range(len(fronts[p])))
    total_dve = sum(back_cost(p, bi)
                    for p in range(len(geom)) for bi in range(n_backs(p)))
    ratio = total_dve / max(total_act, 1.0)

    # backs of the final two rows execute after (or right at) the last
    # tanh: they are tail drain, split across ScalarE+VectorE below, and
    # excluded from the paced in-flight drain
    tail_rows = {len(geom) - 1}
    if len(geom) > 1:
        tail_rows.add(len(geom) - 2)

    pending = []
    act_emitted = 0.0
    dve_emitted = 0.0
    for p in range(len(geom)):
        for fi in range(len(fronts[p])):
            front_mm(p, fi)
            front_tanh(p, fi)
            act_emitted += act_cost(p, fi)
            # cap the drain at 3 backs per front: a section-boundary
            # burst of PE back-matmuls between consecutive A-tile
            # builds would exceed the ACT tile window and starve it
            drained = 0
            while (pending and drained < 3
                   and dve_emitted < act_emitted * ratio):
                bp, bb = pending.pop(0)
                back(bp, bb, False)
                dve_emitted += back_cost(bp, bb)
                drained += 1
        if p not in tail_rows:
            pending.extend((p, bi) for bi in range(n_backs(p)))
    # drain. In the last rep ACT is idle after the final tanh, so
    # alternating evacuations across ScalarE+VectorE halves that tail;
    # in earlier reps the next rep's tanhs already queue on ACT, so
    # scalar copies would lengthen the bottleneck queue — keep them off.
    tail = pending + [(p, bi) for p in sorted(tail_rows)
                      for bi in range(n_backs(p))]
    for i, (bp, bb) in enumerate(tail):
        back(bp, bb, last_rep and i % 2 == 1)


def _host_pred_bias(targets_b, emb, pw1, pb1, pw2, pb2, jw1, jb1):
    """bias[u, j] = (pred @ jw1[E:] + jb1)[u, j] for the U1 joint positions."""
    ext = np.concatenate([np.full(H, BLANK, np.int64), targets_b.astype(np.int64)])
    e = np.concatenate([emb[ext[1:U1 + 1]], emb[ext[0:U1]]], axis=1)  # [101, 512]
    h = np.tanh(e @ pw1 + pb1)
    pred = np.tanh(h @ pw2 + pb2)
    return (pred @ jw1[E:] + jb1).astype(np.float32)  # [101, 128]


def _make_sel(wrow, u0, ws):
    """Selection rhs for a section: K rows = [16 ep rows ; wrow bias rows];
    column (tl, v) sums ep row tl and bias row u0+v."""
    sel = np.zeros((SPAN + wrow, SPAN * ws), np.float16)
    for tl in range(SPAN):
        sel[tl, tl * ws:(tl + 1) * ws] = 1.0
        sel[SPAN + u0:SPAN + u0 + ws, tl * ws:(tl + 1) * ws] += \
            np.eye(ws, dtype=np.float16)
    return sel


def _make_in_maps(encoder_states, targets, emb, pw1, pb1, pw2, pb2, jw1, jb1,
                  jw2, U1e, rows):
    encoder_states = np.asarray(encoder_states, dtype=np.float32)
    jw1 = np.asarray(jw1, dtype=np.float32)
    jw2_np = np.ascontiguousarray(np.asarray(jw2, dtype=np.float32)).astype(np.float16)
    jw1enc = np.ascontiguousarray(jw1[:E])

    # host: encoder projection (fp32 GEMM, zero-padded to span multiple)
    Tpad = T + SPAN
    eproj = np.zeros((B, Tpad, J), np.float16)
    for b in range(B):
        eproj[b, :T] = (encoder_states[b] @ jw1enc).astype(np.float16)
    bias_all = np.empty((B, U1, J), np.float16)
    for b in range(B):
        bias_all[b] = _host_pred_bias(
            np.asarray(targets[b]), np.asarray(emb, np.float32),
            np.asarray(pw1, np.float32), np.asarray(pb1, np.float32),
            np.asarray(pw2, np.float32), np.asarray(pb2, np.float32),
            jw1, np.asarray(jb1, np.float32),
        ).astype(np.float16)

    secs = _sections(rows)
    sel_keys = []
    for (_r, _TB, Wr, u0, Ws, _vf) in secs:
        if (Wr, u0, Ws) not in sel_keys:
            sel_keys.append((Wr, u0, Ws))
    sels = {k: _make_sel(*k) for k in sel_keys}

    in_maps = []
    for c in range(N_CORES):
        m = {"jw2d": jw2_np}
        for k in sel_keys:
            m[f"selw{k[0]}_{k[1]}_{k[2]}"] = sels[k]
        for r, (TBp, Wp, _vf, assign) in enumerate(rows):
            KC = SPAN + Wp
            combd = np.zeros((KC, (TBp // SPAN) * J), np.float16)
            for k, (b, ts) in enumerate(assign[c]):
                if b < 0:
                    continue
                combd[0:SPAN, k * J:(k + 1) * J] = eproj[b, ts:ts + SPAN]
                combd[SPAN:KC, k * J:(k + 1) * J] = bias_all[b, 0:Wp]
            m[f"combd{r}"] = combd
        in_maps.append(m)
    return in_maps


def kernel(encoder_states, encoder_states_size, targets, targets_size,
           emb, pw1, pb1, pw2, pb2, jw1, jb1, jw2, jb2):
    tsz = np.asarray(encoder_states_size).astype(np.int64)
    usz = np.asarray(targets_size).astype(np.int64)
    plan = _plan(tsz, usz)
    if plan is None:  # no valid lattice positions anywhere
        return np.zeros((B, T, U1, V), np.float32)
    U1e, rows = plan
    geom = tuple(_sections(rows))

    if _CACHE.get("geom") != geom:
        _CACHE.clear()
        _CACHE["geom"] = geom
    if "nc" not in _CACHE:
        _CACHE["nc"] = _build_program(reps=1, geom=geom)
    nc = _CACHE["nc"]

    in_maps = _make_in_maps(encoder_states, targets, emb, pw1, pb1, pw2, pb2,
                            jw1, jb1, jw2, U1e, rows)
    _CACHE["in_maps"] = in_maps
    res = run_bass_kernel_spmd(nc, in_maps, core_ids=list(range(N_CORES)))

    jb2f = np.asarray(jb2, np.float32)
    usz1 = usz + 1
    out = np.zeros((B, T, U1, V), np.float32)
    for c in range(N_CORES):
        for p, (r, TBp, _Wr, u0, Ws, vf) in enumerate(geom):
            assign = rows[r][3]
            dev = res.results[c][f"out{p}"]
            if vf:
                dev = dev.reshape(V, TBp, Ws).transpose(1, 2, 0)  # [t, u, v]
            else:
                dev = dev.reshape(TBp, Ws, V)
            for k, (b, ts) in enumerate(assign[c]):
                if b < 0:
                    continue
                nt = min(SPAN, int(tsz[b]) - ts)
                w = min(u0 + Ws, int(usz1[b])) - u0
                if w <= 0:
                    continue
                blk = dev[k * SPAN:k * SPAN + nt, :w, :]
                out[b, ts:ts + nt, u0:u0 + w, :] = \
                    blk.astype(np.float32) + jb2f
    return out
